# revision 6
# baseline (speedup 1.0000x reference)
"""GCN layer (gather + scale + segment-sum + 128x128 matmul) on 8 TRN2 NeuronCores.

Sharding: nodes (and their incident edges, partitioned by dst) are sharded
across the 8 cores; the 128x128 weight is replicated. Per core:

  host (integer/permutation preprocessing only):
    - select edges with dst in the core's 6250-row slice, sort by
      (src>=32768, dst_block) so each 128-edge tile maps to one 128-dst block
      and gather indices fit int16 (low/high base split)
    - pad each (phase, block) edge group to a multiple of 128 with
      (idx=0, w=0, dst_local=-1); tile counts are maxed across cores so all
      8 cores run one SPMD program
    - ship per-edge out-degree counts / per-node in-degree counts (integer
      index bookkeeping); all float math happens on device

  device:
    - s_e   = edge_w * rsqrt(outdeg[src_e])                (DVE/ACT)
    - batched dma_gather of feat rows by src (512B rows)   (SWDGE)
    - fp16 cast of gathered messages                       (ACT)
    - selector tile sel[e, d] = (iota[d]==dst_local[e]) * s_e  (one DVE op)
    - aggT[f, d] += msg_tile^T @ sel_tile  accumulated in PSUM  (PE)
    - rst = (aggT^T @ W) * rsqrt(max(indeg,1)) + b         (PE + ACT + DVE)
"""

import os
import numpy as np

N_NODES = 50000
N_EDGES = 800000
F = 128
C = 8
NPC = N_NODES // C          # 6250 nodes per core
NB = (NPC + 127) // 128     # 49 dst blocks per core (48 full + 106)
SPLIT = 32768               # int16 gather-index base split
CHUNK_T = 8                 # gather chunk size in 128-edge tiles (<=8: 1024-idx ucode packet limit)
CAST_T = 8                  # fp16-cast granularity in tiles


def _host_prep(feat, W, b, edge_w, edge_src, edge_dst):
    src = np.ascontiguousarray(np.asarray(edge_src)).astype(np.int64)
    dst = np.ascontiguousarray(np.asarray(edge_dst)).astype(np.int64)
    w = np.ascontiguousarray(np.asarray(edge_w)).astype(np.float32)

    outcnt = np.bincount(src, minlength=N_NODES)

    per_core = []
    core_of = dst // NPC
    for c in range(C):
        m = core_of == c
        s_c = src[m]
        d_c = dst[m] - c * NPC
        w_c = w[m]
        blk = d_c >> 7
        hi = (s_c >= SPLIT).astype(np.int64)
        perm = np.lexsort((blk, hi))
        per_core.append((s_c[perm], d_c[perm], w_c[perm], blk[perm], hi[perm]))

    cnts = np.zeros((C, 2, NB), np.int64)
    for c in range(C):
        _, _, _, blk, hi = per_core[c]
        for p in range(2):
            cnts[c, p] = np.bincount(blk[hi == p], minlength=NB)
    T_pb = np.maximum(1, (cnts.max(axis=0) + 127) // 128)  # [2, NB]
    off = np.zeros((2, NB), np.int64)
    cur = 0
    for p in range(2):
        for bk in range(NB):
            off[p, bk] = cur
            cur += T_pb[p, bk]
    T_total = cur

    in_maps = []
    for c in range(C):
        s_c, d_c, w_c, blk, hi = per_core[c]
        gidx = np.zeros(T_total * 128, np.int64)
        wv = np.zeros(T_total * 128, np.float32)
        dstl = np.full(T_total * 128, -1, np.int64)
        cnt = np.ones(T_total * 128, np.int64)
        # per (phase, block) the edges are contiguous after the lexsort;
        # compute group start offsets instead of boolean masks per group
        order_key = hi * NB + blk
        grp_cnt = np.bincount(order_key, minlength=2 * NB)
        grp_start = np.concatenate([[0], np.cumsum(grp_cnt)])
        for p in range(2):
            for bk in range(NB):
                g = p * NB + bk
                e0, e1 = grp_start[g], grp_start[g + 1]
                k = e1 - e0
                s0 = off[p, bk] * 128
                gidx[s0:s0 + k] = s_c[e0:e1] - p * SPLIT
                wv[s0:s0 + k] = w_c[e0:e1]
                dstl[s0:s0 + k] = d_c[e0:e1] - bk * 128
                cnt[s0:s0 + k] = outcnt[s_c[e0:e1]]
        icnt = np.bincount(d_c, minlength=NPC)
        icnt_pad = np.ones(NB * 128, np.int64)
        icnt_pad[:NPC] = icnt

        TLs = int(T_pb[0].sum())
        THs = int(T_pb[1].sum())

        def wrap_idx(a):  # [n*128] -> [128, n*8] int16 (16-part wrap, 8x replicated)
            t = a.reshape(-1, 16).T.astype(np.int16)         # [16, n*8]
            return np.ascontiguousarray(np.tile(t, (8, 1)))  # [128, n*8]

        in_maps.append({
            "feat": np.ascontiguousarray(np.asarray(feat, np.float32)),
            "Wm": np.ascontiguousarray(np.asarray(W, np.float32)),
            "bv": np.ascontiguousarray(np.asarray(b, np.float32).reshape(1, F)),
            "wv": np.ascontiguousarray(wv.reshape(T_total, 128).T),
            "dstl": np.ascontiguousarray(dstl.reshape(T_total, 128).T.astype(np.float32)),
            "ocnt": np.ascontiguousarray(cnt.reshape(T_total, 128).T.astype(np.int16)),
            "icnt": np.ascontiguousarray(icnt_pad.reshape(NB, 128).T.astype(np.int16)),
            "idxL": wrap_idx(gidx[:TLs * 128]),
            "idxH": wrap_idx(gidx[TLs * 128:]),
        })
    return T_pb, off, in_maps


_BUILD_CACHE = {}


def _build_program(T_pb, off):
    import concourse.bacc as bacc
    import concourse.bass as bass
    import concourse.mybir as mybir
    import concourse.tile as tile
    from concourse._compat import get_trn_type

    dt = mybir.dt
    AF = mybir.ActivationFunctionType
    ALU = mybir.AluOpType

    TLs = int(T_pb[0].sum())
    THs = int(T_pb[1].sum())
    T_total = TLs + THs

    nc = bacc.Bacc(get_trn_type() or "TRN2", target_bir_lowering=False, debug=False)

    feat_d = nc.dram_tensor("feat", [N_NODES, F], dt.float32, kind="ExternalInput")
    W_d = nc.dram_tensor("Wm", [F, F], dt.float32, kind="ExternalInput")
    b_d = nc.dram_tensor("bv", [1, F], dt.float32, kind="ExternalInput")
    wv_d = nc.dram_tensor("wv", [128, T_total], dt.float32, kind="ExternalInput")
    dstl_d = nc.dram_tensor("dstl", [128, T_total], dt.float32, kind="ExternalInput")
    ocnt_d = nc.dram_tensor("ocnt", [128, T_total], dt.int16, kind="ExternalInput")
    icnt_d = nc.dram_tensor("icnt", [128, NB], dt.int16, kind="ExternalInput")
    idxL_d = nc.dram_tensor("idxL", [128, TLs * 8], dt.int16, kind="ExternalInput")
    idxH_d = nc.dram_tensor("idxH", [128, THs * 8], dt.int16, kind="ExternalInput")
    out_d = nc.dram_tensor("out", [NPC, F], dt.float32, kind="ExternalOutput")

    with tile.TileContext(nc) as tc:
        with (
            tc.tile_pool(name="const", bufs=1) as cpool,
            tc.tile_pool(name="gbuf", bufs=6) as gpool,
            tc.tile_pool(name="mbuf", bufs=6) as mpool,
            tc.tile_pool(name="sel", bufs=8) as spool,
            tc.tile_pool(name="rst", bufs=3) as rpool,
            tc.tile_pool(name="pacc", bufs=3, space="PSUM") as papool,
            tc.tile_pool(name="prst", bufs=2, space="PSUM") as prpool,
        ):
            # ---- constant / setup loads ----
            w_sb = cpool.tile([128, T_total], dt.float32)
            dstl_sb = cpool.tile([128, T_total], dt.float32)
            ocnt_sb = cpool.tile([128, T_total], dt.int16)
            icnt_sb = cpool.tile([128, NB], dt.int16)
            idxL_sb = cpool.tile([128, TLs * 8], dt.int16)
            idxH_sb = cpool.tile([128, THs * 8], dt.int16)
            W_sb = cpool.tile([128, F], dt.float32)
            W_h = cpool.tile([128, F], dt.float16)
            b_sb = cpool.tile([1, F], dt.float32)
            ones1 = cpool.tile([1, F], dt.float32)
            b_bcast = cpool.tile([128, F], dt.float32)
            iota_h = cpool.tile([128, 128], dt.float16)
            s_h = cpool.tile([128, T_total], dt.float32)
            rs_in = cpool.tile([128, NB], dt.float32)
            agg = cpool.tile([128, NB * 128], dt.float32)
            aggTh = cpool.tile([128, NB * 128], dt.float16)
            tmp_f = cpool.tile([128, T_total], dt.float32)
            tmp_g = cpool.tile([128, T_total], dt.float32)
            tmp_i = cpool.tile([128, NB], dt.float32)
            tmp_j = cpool.tile([128, NB], dt.float32)

            nc.sync.dma_start(out=w_sb[:], in_=wv_d[:])
            nc.sync.dma_start(out=dstl_sb[:], in_=dstl_d[:])
            nc.sync.dma_start(out=ocnt_sb[:], in_=ocnt_d[:])
            nc.sync.dma_start(out=icnt_sb[:], in_=icnt_d[:])
            nc.sync.dma_start(out=idxL_sb[:], in_=idxL_d[:])
            nc.sync.dma_start(out=idxH_sb[:], in_=idxH_d[:])
            nc.sync.dma_start(out=W_sb[:], in_=W_d[:])
            nc.sync.dma_start(out=b_sb[:], in_=b_d[:])

            # W in fp16 for the PE
            nc.scalar.activation(out=W_h[:], in_=W_sb[:], func=AF.Copy)

            # broadcast b across partitions via a K=1 outer-product matmul
            nc.vector.memset(ones1[:], 1.0)
            pb = prpool.tile([128, F], dt.float32, space="PSUM")
            nc.tensor.matmul(pb[:], ones1[:], b_sb[:], start=True, stop=True)
            nc.vector.tensor_copy(out=b_bcast[:], in_=pb[:])

            # iota row 0..127 along the free dim (same on every partition)
            nc.gpsimd.iota(
                iota_h[:], pattern=[[1, 128]], base=0, channel_multiplier=0,
                allow_small_or_imprecise_dtypes=True,
            )

            # s_e = w_e * rsqrt(outdeg_e)
            nc.vector.tensor_copy(out=tmp_f[:], in_=ocnt_sb[:])
            nc.vector.reciprocal(out=tmp_g[:], in_=tmp_f[:])
            nc.scalar.activation(out=tmp_f[:], in_=tmp_g[:], func=AF.Sqrt)
            nc.vector.tensor_tensor(out=s_h[:], in0=w_sb[:], in1=tmp_f[:], op=ALU.mult)

            # rs_in = rsqrt(max(indeg, 1))
            nc.vector.tensor_copy(out=tmp_i[:], in_=icnt_sb[:])
            nc.vector.tensor_scalar_max(tmp_j[:], tmp_i[:], 1.0)
            nc.vector.reciprocal(out=tmp_i[:], in_=tmp_j[:])
            nc.scalar.activation(out=rs_in[:], in_=tmp_i[:], func=AF.Sqrt)

            # ---- main aggregation: phase L then phase H ----
            for p in range(2):
                Tp = TLs if p == 0 else THs
                idx_sb = idxL_sb if p == 0 else idxH_sb
                src_ap = feat_d[:, :] if p == 0 else feat_d[SPLIT:, :]
                # block id for each tile of this phase
                tile_blk = np.repeat(np.arange(NB), T_pb[p])
                tile_in_blk = np.concatenate([np.arange(T_pb[p][bk]) for bk in range(NB)])
                pa = None
                for c0 in range(0, Tp, CHUNK_T):
                    ct = min(CHUNK_T, Tp - c0)
                    gbuf = gpool.tile([128, CHUNK_T * 128], dt.float32, tag="gbuf")
                    mbuf = mpool.tile([128, CHUNK_T * 128], dt.float16, tag="mbuf")
                    nc.gpsimd.dma_gather(
                        gbuf[:, :ct * 128].rearrange("p (t e) -> p t e", e=128),
                        src_ap,
                        idx_sb[:, c0 * 8:(c0 + ct) * 8],
                        ct * 128,
                        ct * 128,
                        128,
                    )
                    for s0 in range(0, ct, CAST_T):
                        s1 = min(s0 + CAST_T, ct)
                        nc.scalar.activation(
                            out=mbuf[:, s0 * 128:s1 * 128],
                            in_=gbuf[:, s0 * 128:s1 * 128],
                            func=AF.Copy,
                        )
                    for t in range(ct):
                        gt = c0 + t
                        bk = int(tile_blk[gt])
                        ti = int(tile_in_blk[gt])
                        last = ti == T_pb[p][bk] - 1
                        g = p * TLs + gt
                        sel = spool.tile([128, 128], dt.float16, tag="sel")
                        nc.vector.tensor_scalar(
                            sel[:],
                            iota_h[:],
                            dstl_sb[:, g:g + 1],
                            s_h[:, g:g + 1],
                            op0=ALU.is_equal,
                            op1=ALU.mult,
                        )
                        if ti == 0:
                            pa = papool.tile([128, 128], dt.float32, space="PSUM", tag="pa")
                        nc.tensor.matmul(
                            pa[:],
                            mbuf[:, t * 128:(t + 1) * 128],
                            sel[:],
                            start=(ti == 0),
                            stop=last,
                        )
                        if last:
                            bs = slice(bk * 128, (bk + 1) * 128)
                            if p == 0:
                                nc.scalar.activation(out=agg[:, bs], in_=pa[:], func=AF.Copy)
                            else:
                                nc.vector.tensor_tensor(
                                    out=aggTh[:, bs], in0=pa[:], in1=agg[:, bs], op=ALU.add,
                                )

            # ---- finalize: rst = (aggT^T @ W) * rs_in + b ----
            for bk in range(NB):
                bs = slice(bk * 128, (bk + 1) * 128)
                pr = prpool.tile([128, F], dt.float32, space="PSUM", tag="pr")
                nc.tensor.matmul(pr[:], aggTh[:, bs], W_h[:], start=True, stop=True)
                rt = rpool.tile([128, F], dt.float32, tag="rt")
                nc.scalar.activation(
                    out=rt[:], in_=pr[:], func=AF.Copy, scale=rs_in[:, bk:bk + 1],
                )
                nc.vector.tensor_tensor(out=rt[:], in0=rt[:], in1=b_bcast[:], op=ALU.add)
                n0 = bk * 128
                n1 = min(n0 + 128, NPC)
                nc.sync.dma_start(out=out_d[n0:n1, :], in_=rt[:n1 - n0, :])

    nc.compile()
    return nc


def kernel(feat, W, b, edge_w, edge_src, edge_dst):
    from concourse.bass_utils import run_bass_kernel_spmd

    T_pb, off, in_maps = _host_prep(feat, W, b, edge_w, edge_src, edge_dst)

    key = (tuple(T_pb[0]), tuple(T_pb[1]))
    if key not in _BUILD_CACHE:
        _BUILD_CACHE[key] = _build_program(T_pb, off)
    nc = _BUILD_CACHE[key]

    trace = bool(int(os.environ.get("GCN_TRACE", "0")))
    res = run_bass_kernel_spmd(
        nc, in_maps, core_ids=list(range(C)),
        trace=trace,
        trace_cores=list(range(C)) if trace else None,
    )
    kernel.last_results = res
    out = np.concatenate([r["out"] for r in res.results], axis=0)
    return out.astype(np.float32)
